# revision 2
# baseline (speedup 1.0000x reference)
"""Trainium2 Bass kernel for nn_DMGAGRUcell — v2 (DMA-paced rewrite).

Per core (2 batches b0/b1):
- merged [66]-contraction projections/direct terms (one matmul per chunk),
- fp8-DoubleRow streams; gconv2 (c) streams flipped to node-major
  (stationary = S/adp chunk, moving = y projection, out free = 64) which
  halves their PE cost,
- r|u sigmoid computed in one Act op per [128,512] PSUM slice,
- final gates in node-major with PE-transposed hx and u,
- b1's adp DMA'd in ru-slice chunks so its ru/sigma/y/c pipeline tracks
  DMA arrival; out DMAs naturally queue behind all input transfers.
"""

import numpy as np
import ml_dtypes

BF16 = ml_dtypes.bfloat16
FP8 = ml_dtypes.float8_e4m3fn

N = 2048
B = 16
D_IN = 2
UNITS = 64
F = 66
B_LOC = 2
N_CORES = 8
KC = 16
KP = 8
NS = 4

S_SCALE = 256.0
A_SCALE = 32768.0
LAM = 32768.0
LAM_RU = 8192.0

_CACHE = {}


def _build():
    if "nc" in _CACHE:
        return _CACHE["nc"]

    from contextlib import ExitStack
    import concourse.mybir as mybir
    import concourse.tile as tile
    from concourse import bacc

    f32 = mybir.dt.float32
    bf = mybir.dt.bfloat16
    f8 = mybir.dt.float8e4
    AF = mybir.ActivationFunctionType
    DR = mybir.MatmulPerfMode.DoubleRow

    nc = bacc.Bacc("TRN2", target_bir_lowering=False, debug=False,
                   num_devices=N_CORES)

    adp0_d = nc.dram_tensor("adp0T", [KP, 128, 2, N], f8, kind="ExternalInput")
    # b1 adp pre-permuted to [s, p, kp, r, col] slice-chunk tiles
    adp1_d = nc.dram_tensor("adp1T", [NS, 128, KP, 2, 512], f8,
                            kind="ExternalInput")
    s_d = nc.dram_tensor("sT", [KP, 128, 2, N], f8, kind="ExternalInput")
    hxi_d = nc.dram_tensor("hxi", [B_LOC, F, N], bf, kind="ExternalInput")
    # cols 576:640 hold a stacked identity for PE transposes
    wb_d = nc.dram_tensor("wblob", [128, 640], bf, kind="ExternalInput")
    # node-major output: [b, p, k, ch] with node = 128*k + p
    out_d = nc.dram_tensor("outT", [B_LOC, 128, KC, UNITS], bf,
                           kind="ExternalOutput")

    with tile.TileContext(nc) as tc, ExitStack() as ctx:
        sb = ctx.enter_context(tc.tile_pool(name="sb", bufs=1))
        pp = ctx.enter_context(tc.tile_pool(name="pp", bufs=2, space="PSUM"))

        def ck(k):
            return slice(128 * k, 128 * (k + 1))

        def sl(s):
            return slice(512 * s, 512 * (s + 1))

        # ---- SBUF tiles ----
        hxi = [sb.tile([F, N], bf, tag=f"hxi{b}", name=f"hxi{b}")
               for b in range(B_LOC)]
        rhx = [sb.tile([F, N], bf, tag=f"rhx{b}", name=f"rhx{b}")
               for b in range(B_LOC)]
        wbt = sb.tile([128, 640], bf, tag="wb", name="wbt")
        wb = wbt[0:F, 0:576]
        ident = wbt[:, 576:640]
        s2 = [sb.tile([128, 2, N], f8, tag=f"s{kp}", name=f"s{kp}")
              for kp in range(KP)]
        a2b0 = [sb.tile([128, 2, N], f8, tag=f"a0_{kp}", name=f"a0_{kp}")
                for kp in range(KP)]
        # b1 adp slice-chunk tiles: s0..s2 full, s3 in two kp-halves
        a2b1s = [sb.tile([128, KP, 2, 512], f8, tag=f"a1_{s}", name=f"a1_{s}")
                 for s in range(3)]
        a2b1h = [sb.tile([128, 7, 2, 512], f8, tag="a1h0", name="a1h0"),
                 sb.tile([128, 1, 2, 512], f8, tag="a1h1", name="a1h1")]
        z1 = [sb.tile([128, KC, 128], f8, tag=f"z1_{b}", name=f"z1_{b}")
              for b in range(B_LOC)]
        z2 = [sb.tile([128, KC, 128], f8, tag=f"z2_{b}", name=f"z2_{b}")
              for b in range(B_LOC)]
        y1 = [sb.tile([128, KC, UNITS], f8, tag=f"y1_{b}", name=f"y1_{b}")
              for b in range(B_LOC)]
        y2 = [sb.tile([128, KC, UNITS], f8, tag=f"y2_{b}", name=f"y2_{b}")
              for b in range(B_LOC)]
        ract = [sb.tile([UNITS, N], bf, tag=f"ract{b}", name=f"ract{b}")
                for b in range(B_LOC)]
        u64 = [sb.tile([UNITS, N], bf, tag=f"u64{b}", name=f"u64{b}")
               for b in range(B_LOC)]
        cT = [sb.tile([128, KC, UNITS], bf, tag=f"cT{b}", name=f"cT{b}")
              for b in range(B_LOC)]
        hxT = [sb.tile([128, KC, UNITS], bf, tag=f"hxT{b}", name=f"hxT{b}")
               for b in range(B_LOC)]
        outn = [sb.tile([128, KC, UNITS], bf, tag=f"outn{b}", name=f"outn{b}")
                for b in range(B_LOC)]
        uTp = [None, None]  # u transposes stay in PSUM (bf16), read by gates

        # ---- input DMAs (SP queue; issue order = transfer order) ----
        nc.sync.dma_start(hxi[0][:], hxi_d[0])
        nc.sync.dma_start(wbt[:], wb_d[:])
        nc.sync.dma_start(hxi[1][:], hxi_d[1])
        for kp in range(KP):
            nc.sync.dma_start(s2[kp][:], s_d[kp])
        for kp in range(KP):
            nc.sync.dma_start(a2b0[kp][:], adp0_d[kp])
        for s in range(3):
            nc.sync.dma_start(a2b1s[s][:], adp1_d[s])
        nc.sync.dma_start(a2b1h[0][:], adp1_d[3, :, 0:7])
        nc.sync.dma_start(a2b1h[1][:], adp1_d[3, :, 7:8])
        # rhx input rows copied on-device (keeps the DMA stream dense)
        nc.vector.tensor_copy(rhx[0][UNITS:F, :], hxi[0][UNITS:F, :])
        nc.vector.tensor_copy(rhx[1][UNITS:F, :], hxi[1][UNITS:F, :])

        dum = sb.tile([1, 2], f32, tag="dum", name="dum")
        nc.scalar.activation(dum[0:1, 0:1], wb[0:1, 0:1], AF.Sigmoid)

        dve_cp = nc.vector.tensor_copy
        pool_cp = nc.gpsimd.tensor_copy

        def act_cp(dst, src):
            nc.scalar.activation(dst, src, AF.Copy)

        # =========== phase Z: projections + direct terms ===========
        def zproj(b, col, pstag):
            lo = pp.tile([128, 8, 128], f32, tag=pstag, name=f"zp{b}_{col}_lo")
            hi = pp.tile([128, 8, 128], f32, tag=pstag, name=f"zp{b}_{col}_hi")
            for k in range(KC):
                dst = (lo if k < 8 else hi)[:, k % 8, :]
                nc.tensor.matmul(dst, hxi[b][:, ck(k)], wb[:, col:col + 128],
                                 start=(k % 4 == 0), stop=(k % 4 == 3))
            return lo, hi

        def zdrain(zdst, lo, hi, e1, e2):
            e1(zdst[:, 0:8, :], lo[:])
            e2(zdst[:, 8:16, :], hi[:])

        l0, h0 = zproj(0, 128, "A")
        zdrain(z1[0], l0, h0, dve_cp, act_cp)
        l1, h1 = zproj(0, 256, "B")
        zdrain(z2[0], l1, h1, pool_cp, dve_cp)
        l2, h2 = zproj(1, 128, "A")
        zdrain(z1[1], l2, h2, act_cp, dve_cp)
        l3, h3 = zproj(1, 256, "B")
        zdrain(z2[1], l3, h3, pool_cp, act_cp)

        # ru PSUM: per batch two tiles of 2 bank-aligned slices each
        ruP = [[pp.tile([128, 2, 512], f32, tag=("A" if b == 0 else "B"),
                        name=f"ruP{b}_{h}") for h in range(2)]
               for b in range(B_LOC)]

        def ru_slice(b, s):
            return ruP[b][s // 2][:, s % 2, :]

        for b in range(B_LOC):
            for s in range(NS):
                nc.tensor.matmul(ru_slice(b, s), wb[:, 0:128],
                                 hxi[b][:, sl(s)], start=True, stop=False)

        # ru S-streams (paced by s2 kp arrival)
        for kp in range(KP):
            for b in range(B_LOC):
                for s in range(NS):
                    nc.tensor.matmul(ru_slice(b, s),
                                     z1[b][:, 2 * kp:2 * kp + 2, :],
                                     s2[kp][:, :, sl(s)],
                                     start=False, stop=False, perf_mode=DR)

        # ru adp-stream b0 (paced by a2b0 kp arrival)
        for kp in range(KP):
            for s in range(NS):
                nc.tensor.matmul(ru_slice(0, s),
                                 z2[0][:, 2 * kp:2 * kp + 2, :],
                                 a2b0[kp][:, :, sl(s)],
                                 start=False, stop=(kp == KP - 1), perf_mode=DR)

        # ---- gconv2 helpers ----
        psYa = [None, None]   # slices s0,s1: [128, 2, 2, 4, 64]
        psYb = [None, None]   # slices s2,s3
        psC = [None, None]
        c_started = [[False, False], [False, False]]

        def sigma(b, s):
            nc.scalar.activation(ru_act[b][:, sl(s)], ru_slice(b, s),
                                 AF.Sigmoid, scale=1.0 / LAM_RU)

        def rhx_mul(b, s):
            nc.vector.tensor_mul(rhx[b][0:UNITS, sl(s)],
                                 ru_act[b][0:UNITS, sl(s)],
                                 hxi[b][0:UNITS, sl(s)])

        def yproj_slice(b, s):
            t = psYa[b] if s < 2 else psYb[b]
            si = s % 2
            for c in range(4):
                nc.tensor.matmul(t[:, si, 0, c, :], rhx[b][:, ck(4 * s + c)],
                                 wb[:, 384:448], start=(c == 0), stop=False)
            for c in range(4):
                nc.tensor.matmul(t[:, si, 1, c, :], rhx[b][:, ck(4 * s + c)],
                                 wb[:, 448:512], start=False, stop=(c == 3))

        def ydrain_slice(b, s, e1, e2):
            t = psYa[b] if s < 2 else psYb[b]
            si = s % 2
            e1(y1[b][:, 4 * s:4 * s + 4, :], t[:, si, 0, :, :])
            e2(y2[b][:, 4 * s:4 * s + 4, :], t[:, si, 1, :, :])

        def c_direct(b, dlist):
            for d in dlist:
                bank = d // 8
                st = not c_started[b][bank]
                c_started[b][bank] = True
                nc.tensor.matmul(psC[b][:, d, :], rhx[b][:, ck(d)],
                                 wb[:, 512:576], start=st, stop=False)

        def a1_kp(kp, s):
            # a2b1 chunk [128, 2, 512] for source block kp, ru slice s
            if s < 3:
                return a2b1s[s][:, kp, :, :]
            if kp < 7:
                return a2b1h[0][:, kp, :, :]
            return a2b1h[1][:, 0, :, :]

        def c_stream_kp(b, kp, stop=False):
            for d in range(KC):
                bank = d // 8
                st = not c_started[b][bank]
                c_started[b][bank] = True
                nc.tensor.matmul(psC[b][:, d, :], s2[kp][:, :, ck(d)],
                                 y1[b][:, 2 * kp:2 * kp + 2, :],
                                 start=st, stop=False, perf_mode=DR)
            for d in range(KC):
                amat = (a2b0[kp] if b == 0
                        else a1_kp(kp, d // 4))[:, :, 128 * (d % 4):
                                                128 * (d % 4) + 128]
                nc.tensor.matmul(psC[b][:, d, :], amat,
                                 y2[b][:, 2 * kp:2 * kp + 2, :],
                                 start=False, stop=(stop and d % 8 == 7),
                                 perf_mode=DR)

        def transpose16(ps, src_rows, idsl=slice(0, UNITS)):
            # 16 chunk transposes [64,128] -> [128,64]; bf16 psum = 1 bank
            for k in range(KC):
                nc.tensor.matmul(ps[:, k, :], src_rows(k), ident[idsl, :],
                                 is_transpose=True,
                                 start=(k == 0), stop=(k == KC - 1))

        def tanh_bank(b, bank):
            nc.scalar.activation(cT[b][:, 8 * bank:8 * bank + 8, :],
                                 psC[b][:, 8 * bank:8 * bank + 8, :],
                                 AF.Tanh, scale=1.0 / LAM)

        def gates_bank(b, bank):
            bsl = slice(8 * bank, 8 * bank + 8)
            nc.vector.tensor_sub(outn[b][:, bsl, :], hxT[b][:, bsl, :],
                                 cT[b][:, bsl, :])
            nc.vector.tensor_mul(outn[b][:, bsl, :], uTp[b][:, bsl, :],
                                 outn[b][:, bsl, :])
            nc.vector.tensor_add(outn[b][:, bsl, :], outn[b][:, bsl, :],
                                 cT[b][:, bsl, :])

        # =========== b0 gconv2 (a2b0 fully resident) ===========
        for s in range(NS):
            sigma(0, s)
            rhx_mul(0, s)
        psYa[0] = pp.tile([128, 2, 2, 4, UNITS], f32, tag="A", name="psYa0")
        psYb[0] = pp.tile([128, 2, 2, 4, UNITS], f32, tag="A", name="psYb0")
        for s in range(NS):
            yproj_slice(0, s)
            ydrain_slice(0, s, dve_cp, (act_cp if s % 2 else dve_cp))
        psC[0] = pp.tile([128, KC, UNITS], f32, tag="A", name="psC0")
        c_direct(0, range(KC))
        for kp in range(KP):
            c_stream_kp(0, kp, stop=(kp == KP - 1))
        tanh_bank(0, 0)
        tanh_bank(0, 1)
        hxTp0 = pp.tile([128, KC, UNITS], bf, tag="A", name="hxTp0")
        transpose16(hxTp0, lambda k: hxi[0][0:UNITS, ck(k)])
        pool_cp(hxT[0][:], hxTp0[:])
        uTp[0] = pp.tile([128, KC, UNITS], bf, tag="A", name="uTp0")
        transpose16(uTp[0], lambda k: ru_act[0][UNITS:128, ck(k)],
                    idsl=slice(UNITS, 128))
        hxTp1 = pp.tile([128, KC, UNITS], bf, tag="A", name="hxTp1")
        transpose16(hxTp1, lambda k: hxi[1][0:UNITS, ck(k)])
        pool_cp(hxT[1][:], hxTp1[:])
        gates_bank(0, 0)
        gates_bank(0, 1)

        # =========== b1: paced by slice-chunk DMA of a2b1 ===========
        def ru1_adp(s, kps, stop=False):
            for kp in kps:
                nc.tensor.matmul(ru_slice(1, s),
                                 z2[1][:, 2 * kp:2 * kp + 2, :],
                                 a1_kp(kp, s),
                                 start=False, stop=(stop and kp == KP - 1),
                                 perf_mode=DR)

        # PE work emitted in data-ready order so ladders pipeline
        ru1_adp(0, range(KP), stop=True)
        sigma(1, 0)
        rhx_mul(1, 0)
        ru1_adp(1, range(KP), stop=True)
        sigma(1, 1)
        rhx_mul(1, 1)

        psYa[1] = pp.tile([128, 2, 2, 4, UNITS], f32, tag="B", name="psYa1")
        psYb[1] = pp.tile([128, 2, 2, 4, UNITS], f32, tag="A", name="psYb1")
        psC[1] = pp.tile([128, KC, UNITS], f32, tag="A", name="psC1")
        yproj_slice(1, 0)
        ydrain_slice(1, 0, dve_cp, pool_cp)
        c_direct(1, range(0, 4))
        yproj_slice(1, 1)
        ydrain_slice(1, 1, dve_cp, pool_cp)
        c_direct(1, range(4, 8))
        c_stream_kp(1, 0)
        c_stream_kp(1, 1)
        c_stream_kp(1, 2)
        c_stream_kp(1, 3)

        ru1_adp(2, range(KP), stop=True)
        sigma(1, 2)
        rhx_mul(1, 2)
        yproj_slice(1, 2)
        ydrain_slice(1, 2, dve_cp, pool_cp)
        c_direct(1, range(8, 12))
        c_stream_kp(1, 4)
        c_stream_kp(1, 5)

        ru1_adp(3, range(0, 7))
        ru1_adp(3, range(7, KP), stop=True)
        sigma(1, 3)
        rhx_mul(1, 3)
        yproj_slice(1, 3)
        ydrain_slice(1, 3, dve_cp, pool_cp)
        c_direct(1, range(12, KC))
        uTp[1] = pp.tile([128, KC, UNITS], bf, tag="B", name="uTp1")
        transpose16(uTp[1], lambda k: ru_act[1][UNITS:128, ck(k)],
                    idsl=slice(UNITS, 128))
        c_stream_kp(1, 6)
        c_stream_kp(1, 7, stop=True)
        tanh_bank(1, 0)
        tanh_bank(1, 1)
        gates_bank(1, 0)
        gates_bank(1, 1)

        # =========== out DMAs (queue behind all input transfers) ===========
        for b in range(B_LOC):
            nc.sync.dma_start(out_d[b, :, 0:8, :], outn[b][:, 0:8, :])
            nc.sync.dma_start(out_d[b, :, 8:16, :], outn[b][:, 8:16, :])

    nc.compile()
    _CACHE["nc"] = nc
    return nc


def _prep_host(inputs, hx, adp, support_rows, support_cols, support_vals,
               W_ru, W_c):
    S = np.zeros((N, N), np.float32)
    np.add.at(S, (support_rows, support_cols), support_vals)
    s2 = np.ascontiguousarray(
        (S.T * S_SCALE).reshape(KP, 2, 128, N).transpose(0, 2, 1, 3)
    ).astype(FP8)
    adp2 = np.ascontiguousarray(
        (adp.transpose(0, 2, 1) * A_SCALE).reshape(B, KP, 2, 128, N)
        .transpose(0, 1, 3, 2, 4)
    ).astype(FP8)

    xcat = np.concatenate(
        [inputs.reshape(B, N, D_IN), hx.reshape(B, N, UNITS)], axis=2)
    xT = np.ascontiguousarray(xcat, dtype=np.float32).transpose(0, 2, 1)
    hxih = np.concatenate([xT[:, D_IN:F, :], xT[:, 0:D_IN, :]], axis=1)
    hxih = np.ascontiguousarray(hxih).astype(BF16)

    wru = W_ru.reshape(F, 3, 2 * UNITS).astype(np.float32)
    wc = W_c.reshape(F, 3, UNITS).astype(np.float32)
    perm = np.concatenate([np.arange(D_IN, F), np.arange(0, D_IN)])
    wblob = np.zeros((128, 640), np.float32)
    wblob[0:F, 0:128] = wru[perm, 0, :] * LAM_RU
    wblob[0:F, 128:256] = wru[perm, 1, :] * (LAM_RU / S_SCALE)
    wblob[0:F, 256:384] = wru[perm, 2, :] * (LAM_RU / A_SCALE)
    wblob[0:F, 384:448] = wc[perm, 1, :] * (LAM / S_SCALE)
    wblob[0:F, 448:512] = wc[perm, 2, :] * (LAM / A_SCALE)
    wblob[0:F, 512:576] = wc[perm, 0, :] * LAM
    wblob[0:UNITS, 576:640] = np.eye(UNITS)
    wblob[UNITS:128, 576:640] = np.eye(UNITS)

    shared = {"sT": s2, "wblob": wblob.astype(BF16)}
    in_maps = []
    for c in range(N_CORES):
        lo = c * B_LOC
        # [kp, p, r, col] -> [s, p, kp, r, 512]
        a1 = adp2[lo + 1].reshape(KP, 128, 2, NS, 512).transpose(3, 1, 0, 2, 4)
        in_maps.append({
            "adp0T": np.ascontiguousarray(adp2[lo]),
            "adp1T": np.ascontiguousarray(a1),
            "hxi": np.ascontiguousarray(hxih[lo:lo + 2]),
            **shared,
        })
    return in_maps


def kernel(inputs, hx, adp, support_rows, support_cols, support_vals,
           W_ru, W_c, time_axis=None):
    from concourse.bass_utils import run_bass_kernel_spmd

    inputs = np.asarray(inputs, dtype=np.float32)
    hx = np.asarray(hx, dtype=np.float32)
    adp = np.asarray(adp, dtype=np.float32)
    support_rows = np.asarray(support_rows)
    support_cols = np.asarray(support_cols)
    support_vals = np.asarray(support_vals, dtype=np.float32)
    W_ru = np.asarray(W_ru, dtype=np.float32)
    W_c = np.asarray(W_c, dtype=np.float32)

    nc = _build()
    in_maps = _prep_host(inputs, hx, adp, support_rows, support_cols,
                         support_vals, W_ru, W_c)

    res = run_bass_kernel_spmd(nc, in_maps, core_ids=list(range(N_CORES)),
                               trace=False)
    _CACHE["last_result"] = res

    out = np.empty((B, N * UNITS), np.float32)
    for c in range(N_CORES):
        outT = np.asarray(res.results[c]["outT"], dtype=np.float32)
        for i in range(B_LOC):
            out[c * B_LOC + i] = np.ascontiguousarray(
                outT[i].transpose(1, 0, 2)).reshape(N * UNITS)
    return out


# revision 3
# speedup vs baseline: 1.0010x; 1.0010x over previous
"""Trainium2 Bass kernel for nn_DMGAGRUcell — v2 (DMA-paced rewrite).

Per core (2 batches b0/b1):
- merged [66]-contraction projections/direct terms (one matmul per chunk),
- fp8-DoubleRow streams; gconv2 (c) streams flipped to node-major
  (stationary = S/adp chunk, moving = y projection, out free = 64) which
  halves their PE cost,
- r|u sigmoid computed in one Act op per [128,512] PSUM slice,
- final gates in node-major with PE-transposed hx and u,
- b1's adp DMA'd in ru-slice chunks so its ru/sigma/y/c pipeline tracks
  DMA arrival; out DMAs naturally queue behind all input transfers.
"""

import numpy as np
import ml_dtypes

BF16 = ml_dtypes.bfloat16
FP8 = ml_dtypes.float8_e4m3fn

N = 2048
B = 16
D_IN = 2
UNITS = 64
F = 66
B_LOC = 2
N_CORES = 8
KC = 16
KP = 8
NS = 4

S_SCALE = 256.0
A_SCALE = 32768.0
LAM = 32768.0
LAM_RU = 8192.0

_CACHE = {}


def _build():
    if "nc" in _CACHE:
        return _CACHE["nc"]

    from contextlib import ExitStack
    import concourse.mybir as mybir
    import concourse.tile as tile
    from concourse import bacc

    f32 = mybir.dt.float32
    bf = mybir.dt.bfloat16
    f8 = mybir.dt.float8e4
    AF = mybir.ActivationFunctionType
    DR = mybir.MatmulPerfMode.DoubleRow

    nc = bacc.Bacc("TRN2", target_bir_lowering=False, debug=False,
                   num_devices=N_CORES)

    adp0_d = nc.dram_tensor("adp0T", [KP, 128, 2, N], f8, kind="ExternalInput")
    # b1 adp pre-permuted to [s, p, kp, r, col] slice-chunk tiles
    adp1_d = nc.dram_tensor("adp1T", [NS, 128, KP, 2, 512], f8,
                            kind="ExternalInput")
    s_d = nc.dram_tensor("sT", [KP, 128, 2, N], f8, kind="ExternalInput")
    hxi_d = nc.dram_tensor("hxi", [B_LOC, F, N], bf, kind="ExternalInput")
    # cols 576:640 hold a stacked identity for PE transposes
    wb_d = nc.dram_tensor("wblob", [128, 640], bf, kind="ExternalInput")
    # node-major output: [b, p, k, ch] with node = 128*k + p
    out_d = nc.dram_tensor("outT", [B_LOC, 128, KC, UNITS], bf,
                           kind="ExternalOutput")

    with tile.TileContext(nc) as tc, ExitStack() as ctx:
        sb = ctx.enter_context(tc.tile_pool(name="sb", bufs=1))
        pp = ctx.enter_context(tc.tile_pool(name="pp", bufs=2, space="PSUM"))

        def ck(k):
            return slice(128 * k, 128 * (k + 1))

        def sl(s):
            return slice(512 * s, 512 * (s + 1))

        # ---- SBUF tiles ----
        hxi = [sb.tile([F, N], bf, tag=f"hxi{b}", name=f"hxi{b}")
               for b in range(B_LOC)]
        rhx = [sb.tile([F, N], bf, tag=f"rhx{b}", name=f"rhx{b}")
               for b in range(B_LOC)]
        wbt = sb.tile([128, 640], bf, tag="wb", name="wbt")
        wb = wbt[0:F, 0:576]
        ident = wbt[:, 576:640]
        s2 = [sb.tile([128, 2, N], f8, tag=f"s{kp}", name=f"s{kp}")
              for kp in range(KP)]
        a2b0 = [sb.tile([128, 2, N], f8, tag=f"a0_{kp}", name=f"a0_{kp}")
                for kp in range(KP)]
        # b1 adp slice-chunk tiles: s0..s2 full, s3 in two kp-halves
        a2b1s = [sb.tile([128, KP, 2, 512], f8, tag=f"a1_{s}", name=f"a1_{s}")
                 for s in range(3)]
        a2b1h = [sb.tile([128, 7, 2, 512], f8, tag="a1h0", name="a1h0"),
                 sb.tile([128, 1, 2, 512], f8, tag="a1h1", name="a1h1")]
        z1 = [sb.tile([128, KC, 128], f8, tag=f"z1_{b}", name=f"z1_{b}")
              for b in range(B_LOC)]
        z2 = [sb.tile([128, KC, 128], f8, tag=f"z2_{b}", name=f"z2_{b}")
              for b in range(B_LOC)]
        y1 = [sb.tile([128, KC, UNITS], f8, tag=f"y1_{b}", name=f"y1_{b}")
              for b in range(B_LOC)]
        y2 = [sb.tile([128, KC, UNITS], f8, tag=f"y2_{b}", name=f"y2_{b}")
              for b in range(B_LOC)]
        ract = [sb.tile([UNITS, N], bf, tag=f"ract{b}", name=f"ract{b}")
                for b in range(B_LOC)]
        u64 = [sb.tile([UNITS, N], bf, tag=f"u64{b}", name=f"u64{b}")
               for b in range(B_LOC)]
        cT = [sb.tile([128, KC, UNITS], bf, tag=f"cT{b}", name=f"cT{b}")
              for b in range(B_LOC)]
        hxT = [sb.tile([128, KC, UNITS], bf, tag=f"hxT{b}", name=f"hxT{b}")
               for b in range(B_LOC)]
        outn = [sb.tile([128, KC, UNITS], bf, tag=f"outn{b}", name=f"outn{b}")
                for b in range(B_LOC)]
        uTp = [None, None]  # u transposes stay in PSUM (bf16), read by gates

        # ---- input DMAs; first via gpsimd SWDGE (faster cold start) ----
        nc.gpsimd.dma_start(hxi[0][:], hxi_d[0])
        nc.sync.dma_start(wbt[:], wb_d[:])
        nc.sync.dma_start(hxi[1][:], hxi_d[1])
        for kp in range(KP):
            nc.sync.dma_start(s2[kp][:], s_d[kp])
        for kp in range(KP):
            nc.sync.dma_start(a2b0[kp][:], adp0_d[kp])
        for s in range(3):
            nc.sync.dma_start(a2b1s[s][:], adp1_d[s])
        nc.sync.dma_start(a2b1h[0][:], adp1_d[3, :, 0:7])
        nc.sync.dma_start(a2b1h[1][:], adp1_d[3, :, 7:8])
        # rhx input rows copied on-device (keeps the DMA stream dense)
        nc.vector.tensor_copy(rhx[0][UNITS:F, :], hxi[0][UNITS:F, :])
        nc.vector.tensor_copy(rhx[1][UNITS:F, :], hxi[1][UNITS:F, :])

        dum = sb.tile([1, 2], f32, tag="dum", name="dum")
        nc.scalar.activation(dum[0:1, 0:1], wb[0:1, 0:1], AF.Sigmoid)

        dve_cp = nc.vector.tensor_copy
        pool_cp = nc.gpsimd.tensor_copy

        def act_cp(dst, src):
            nc.scalar.activation(dst, src, AF.Copy)

        # =========== phase Z: projections + direct terms ===========
        def zproj(b, col, pstag):
            lo = pp.tile([128, 8, 128], f32, tag=pstag, name=f"zp{b}_{col}_lo")
            hi = pp.tile([128, 8, 128], f32, tag=pstag, name=f"zp{b}_{col}_hi")
            for k in range(KC):
                dst = (lo if k < 8 else hi)[:, k % 8, :]
                nc.tensor.matmul(dst, hxi[b][:, ck(k)], wb[:, col:col + 128],
                                 start=(k % 4 == 0), stop=(k % 4 == 3))
            return lo, hi

        def zdrain(zdst, lo, hi, e1, e2):
            e1(zdst[:, 0:8, :], lo[:])
            e2(zdst[:, 8:16, :], hi[:])

        l0, h0 = zproj(0, 128, "A")
        zdrain(z1[0], l0, h0, dve_cp, act_cp)
        l1, h1 = zproj(0, 256, "B")
        zdrain(z2[0], l1, h1, pool_cp, dve_cp)
        l2, h2 = zproj(1, 128, "A")
        zdrain(z1[1], l2, h2, act_cp, dve_cp)
        l3, h3 = zproj(1, 256, "B")
        zdrain(z2[1], l3, h3, pool_cp, act_cp)

        # ru PSUM: per batch two tiles of 2 bank-aligned slices each
        ruP = [[pp.tile([128, 2, 512], f32, tag=("A" if b == 0 else "B"),
                        name=f"ruP{b}_{h}") for h in range(2)]
               for b in range(B_LOC)]

        def ru_slice(b, s):
            return ruP[b][s // 2][:, s % 2, :]

        for b in range(B_LOC):
            for s in range(NS):
                nc.tensor.matmul(ru_slice(b, s), wb[:, 0:128],
                                 hxi[b][:, sl(s)], start=True, stop=False)

        # ru S-streams (paced by s2 kp arrival)
        for kp in range(KP):
            for b in range(B_LOC):
                for s in range(NS):
                    nc.tensor.matmul(ru_slice(b, s),
                                     z1[b][:, 2 * kp:2 * kp + 2, :],
                                     s2[kp][:, :, sl(s)],
                                     start=False, stop=False, perf_mode=DR)

        # ru adp-stream b0 (paced by a2b0 kp arrival)
        for kp in range(KP):
            for s in range(NS):
                nc.tensor.matmul(ru_slice(0, s),
                                 z2[0][:, 2 * kp:2 * kp + 2, :],
                                 a2b0[kp][:, :, sl(s)],
                                 start=False, stop=(kp == KP - 1), perf_mode=DR)

        # ---- gconv2 helpers ----
        psYa = [None, None]   # slices s0,s1: [128, 2, 2, 4, 64]
        psYb = [None, None]   # slices s2,s3
        psC = [None, None]
        c_started = [[False, False], [False, False]]

        def sigma(b, s):
            nc.scalar.activation(ru_act[b][:, sl(s)], ru_slice(b, s),
                                 AF.Sigmoid, scale=1.0 / LAM_RU)

        def rhx_mul(b, s):
            nc.vector.tensor_mul(rhx[b][0:UNITS, sl(s)],
                                 ru_act[b][0:UNITS, sl(s)],
                                 hxi[b][0:UNITS, sl(s)])

        def yproj_slice(b, s):
            t = psYa[b] if s < 2 else psYb[b]
            si = s % 2
            for c in range(4):
                nc.tensor.matmul(t[:, si, 0, c, :], rhx[b][:, ck(4 * s + c)],
                                 wb[:, 384:448], start=(c == 0), stop=False)
            for c in range(4):
                nc.tensor.matmul(t[:, si, 1, c, :], rhx[b][:, ck(4 * s + c)],
                                 wb[:, 448:512], start=False, stop=(c == 3))

        def ydrain_slice(b, s, e1, e2):
            t = psYa[b] if s < 2 else psYb[b]
            si = s % 2
            e1(y1[b][:, 4 * s:4 * s + 4, :], t[:, si, 0, :, :])
            e2(y2[b][:, 4 * s:4 * s + 4, :], t[:, si, 1, :, :])

        def c_direct(b, dlist):
            for d in dlist:
                bank = d // 8
                st = not c_started[b][bank]
                c_started[b][bank] = True
                nc.tensor.matmul(psC[b][:, d, :], rhx[b][:, ck(d)],
                                 wb[:, 512:576], start=st, stop=False)

        def a1_kp(kp, s):
            # a2b1 chunk [128, 2, 512] for source block kp, ru slice s
            if s < 3:
                return a2b1s[s][:, kp, :, :]
            if kp < 7:
                return a2b1h[0][:, kp, :, :]
            return a2b1h[1][:, 0, :, :]

        def c_stream_kp(b, kp, stop=False):
            for d in range(KC):
                bank = d // 8
                st = not c_started[b][bank]
                c_started[b][bank] = True
                nc.tensor.matmul(psC[b][:, d, :], s2[kp][:, :, ck(d)],
                                 y1[b][:, 2 * kp:2 * kp + 2, :],
                                 start=st, stop=False, perf_mode=DR)
            for d in range(KC):
                amat = (a2b0[kp] if b == 0
                        else a1_kp(kp, d // 4))[:, :, 128 * (d % 4):
                                                128 * (d % 4) + 128]
                nc.tensor.matmul(psC[b][:, d, :], amat,
                                 y2[b][:, 2 * kp:2 * kp + 2, :],
                                 start=False, stop=(stop and d % 8 == 7),
                                 perf_mode=DR)

        def transpose16(ps, src_rows, idsl=slice(0, UNITS)):
            # 16 chunk transposes [64,128] -> [128,64]; bf16 psum = 1 bank
            for k in range(KC):
                nc.tensor.matmul(ps[:, k, :], src_rows(k), ident[idsl, :],
                                 is_transpose=True,
                                 start=(k == 0), stop=(k == KC - 1))

        def tanh_bank(b, bank):
            nc.scalar.activation(cT[b][:, 8 * bank:8 * bank + 8, :],
                                 psC[b][:, 8 * bank:8 * bank + 8, :],
                                 AF.Tanh, scale=1.0 / LAM)

        def gates_bank(b, bank):
            bsl = slice(8 * bank, 8 * bank + 8)
            nc.vector.tensor_sub(outn[b][:, bsl, :], hxT[b][:, bsl, :],
                                 cT[b][:, bsl, :])
            nc.vector.tensor_mul(outn[b][:, bsl, :], uTp[b][:, bsl, :],
                                 outn[b][:, bsl, :])
            nc.vector.tensor_add(outn[b][:, bsl, :], outn[b][:, bsl, :],
                                 cT[b][:, bsl, :])

        # =========== b0 gconv2 (a2b0 fully resident) ===========
        for s in range(NS):
            sigma(0, s)
            rhx_mul(0, s)
        psYa[0] = pp.tile([128, 2, 2, 4, UNITS], f32, tag="A", name="psYa0")
        psYb[0] = pp.tile([128, 2, 2, 4, UNITS], f32, tag="A", name="psYb0")
        for s in range(NS):
            yproj_slice(0, s)
            ydrain_slice(0, s, dve_cp, (act_cp if s % 2 else dve_cp))
        psC[0] = pp.tile([128, KC, UNITS], f32, tag="A", name="psC0")
        c_direct(0, range(KC))
        for kp in range(KP):
            c_stream_kp(0, kp, stop=(kp == KP - 1))
        tanh_bank(0, 0)
        tanh_bank(0, 1)
        hxTp0 = pp.tile([128, KC, UNITS], bf, tag="A", name="hxTp0")
        transpose16(hxTp0, lambda k: hxi[0][0:UNITS, ck(k)])
        pool_cp(hxT[0][:], hxTp0[:])
        uTp[0] = pp.tile([128, KC, UNITS], bf, tag="A", name="uTp0")
        transpose16(uTp[0], lambda k: ru_act[0][UNITS:128, ck(k)],
                    idsl=slice(UNITS, 128))
        hxTp1 = pp.tile([128, KC, UNITS], bf, tag="A", name="hxTp1")
        transpose16(hxTp1, lambda k: hxi[1][0:UNITS, ck(k)])
        pool_cp(hxT[1][:], hxTp1[:])
        gates_bank(0, 0)
        gates_bank(0, 1)

        # =========== b1: paced by slice-chunk DMA of a2b1 ===========
        def ru1_adp(s, kps, stop=False):
            for kp in kps:
                nc.tensor.matmul(ru_slice(1, s),
                                 z2[1][:, 2 * kp:2 * kp + 2, :],
                                 a1_kp(kp, s),
                                 start=False, stop=(stop and kp == KP - 1),
                                 perf_mode=DR)

        # PE work emitted in data-ready order so ladders pipeline
        ru1_adp(0, range(KP), stop=True)
        sigma(1, 0)
        rhx_mul(1, 0)
        ru1_adp(1, range(KP), stop=True)
        sigma(1, 1)
        rhx_mul(1, 1)

        psYa[1] = pp.tile([128, 2, 2, 4, UNITS], f32, tag="B", name="psYa1")
        psYb[1] = pp.tile([128, 2, 2, 4, UNITS], f32, tag="A", name="psYb1")
        psC[1] = pp.tile([128, KC, UNITS], f32, tag="A", name="psC1")
        yproj_slice(1, 0)
        ydrain_slice(1, 0, dve_cp, pool_cp)
        c_direct(1, range(0, 4))
        yproj_slice(1, 1)
        ydrain_slice(1, 1, dve_cp, pool_cp)
        c_direct(1, range(4, 8))
        c_stream_kp(1, 0)
        c_stream_kp(1, 1)
        c_stream_kp(1, 2)
        c_stream_kp(1, 3)

        ru1_adp(2, range(KP), stop=True)
        sigma(1, 2)
        rhx_mul(1, 2)
        yproj_slice(1, 2)
        ydrain_slice(1, 2, dve_cp, pool_cp)
        c_direct(1, range(8, 12))
        c_stream_kp(1, 4)
        c_stream_kp(1, 5)

        ru1_adp(3, range(0, 7))
        ru1_adp(3, range(7, KP), stop=True)
        sigma(1, 3)
        rhx_mul(1, 3)
        yproj_slice(1, 3)
        ydrain_slice(1, 3, dve_cp, pool_cp)
        c_direct(1, range(12, KC))
        uTp[1] = pp.tile([128, KC, UNITS], bf, tag="B", name="uTp1")
        transpose16(uTp[1], lambda k: ru_act[1][UNITS:128, ck(k)],
                    idsl=slice(UNITS, 128))
        c_stream_kp(1, 6)
        c_stream_kp(1, 7, stop=True)
        tanh_bank(1, 0)
        tanh_bank(1, 1)
        gates_bank(1, 0)
        gates_bank(1, 1)

        # =========== out DMAs (queue behind all input transfers) ===========
        for b in range(B_LOC):
            nc.sync.dma_start(out_d[b, :, 0:8, :], outn[b][:, 0:8, :])
            nc.sync.dma_start(out_d[b, :, 8:16, :], outn[b][:, 8:16, :])

    nc.compile()
    _CACHE["nc"] = nc
    return nc


def _prep_host(inputs, hx, adp, support_rows, support_cols, support_vals,
               W_ru, W_c):
    S = np.zeros((N, N), np.float32)
    np.add.at(S, (support_rows, support_cols), support_vals)
    s2 = np.ascontiguousarray(
        (S.T * S_SCALE).reshape(KP, 2, 128, N).transpose(0, 2, 1, 3)
    ).astype(FP8)
    adp2 = np.ascontiguousarray(
        (adp.transpose(0, 2, 1) * A_SCALE).reshape(B, KP, 2, 128, N)
        .transpose(0, 1, 3, 2, 4)
    ).astype(FP8)

    xcat = np.concatenate(
        [inputs.reshape(B, N, D_IN), hx.reshape(B, N, UNITS)], axis=2)
    xT = np.ascontiguousarray(xcat, dtype=np.float32).transpose(0, 2, 1)
    hxih = np.concatenate([xT[:, D_IN:F, :], xT[:, 0:D_IN, :]], axis=1)
    hxih = np.ascontiguousarray(hxih).astype(BF16)

    wru = W_ru.reshape(F, 3, 2 * UNITS).astype(np.float32)
    wc = W_c.reshape(F, 3, UNITS).astype(np.float32)
    perm = np.concatenate([np.arange(D_IN, F), np.arange(0, D_IN)])
    wblob = np.zeros((128, 640), np.float32)
    wblob[0:F, 0:128] = wru[perm, 0, :] * LAM_RU
    wblob[0:F, 128:256] = wru[perm, 1, :] * (LAM_RU / S_SCALE)
    wblob[0:F, 256:384] = wru[perm, 2, :] * (LAM_RU / A_SCALE)
    wblob[0:F, 384:448] = wc[perm, 1, :] * (LAM / S_SCALE)
    wblob[0:F, 448:512] = wc[perm, 2, :] * (LAM / A_SCALE)
    wblob[0:F, 512:576] = wc[perm, 0, :] * LAM
    wblob[0:UNITS, 576:640] = np.eye(UNITS)
    wblob[UNITS:128, 576:640] = np.eye(UNITS)

    shared = {"sT": s2, "wblob": wblob.astype(BF16)}
    in_maps = []
    for c in range(N_CORES):
        lo = c * B_LOC
        # [kp, p, r, col] -> [s, p, kp, r, 512]
        a1 = adp2[lo + 1].reshape(KP, 128, 2, NS, 512).transpose(3, 1, 0, 2, 4)
        in_maps.append({
            "adp0T": np.ascontiguousarray(adp2[lo]),
            "adp1T": np.ascontiguousarray(a1),
            "hxi": np.ascontiguousarray(hxih[lo:lo + 2]),
            **shared,
        })
    return in_maps


def kernel(inputs, hx, adp, support_rows, support_cols, support_vals,
           W_ru, W_c, time_axis=None):
    from concourse.bass_utils import run_bass_kernel_spmd

    inputs = np.asarray(inputs, dtype=np.float32)
    hx = np.asarray(hx, dtype=np.float32)
    adp = np.asarray(adp, dtype=np.float32)
    support_rows = np.asarray(support_rows)
    support_cols = np.asarray(support_cols)
    support_vals = np.asarray(support_vals, dtype=np.float32)
    W_ru = np.asarray(W_ru, dtype=np.float32)
    W_c = np.asarray(W_c, dtype=np.float32)

    nc = _build()
    in_maps = _prep_host(inputs, hx, adp, support_rows, support_cols,
                         support_vals, W_ru, W_c)

    res = run_bass_kernel_spmd(nc, in_maps, core_ids=list(range(N_CORES)),
                               trace=False)
    _CACHE["last_result"] = res

    out = np.empty((B, N * UNITS), np.float32)
    for c in range(N_CORES):
        outT = np.asarray(res.results[c]["outT"], dtype=np.float32)
        for i in range(B_LOC):
            out[c * B_LOC + i] = np.ascontiguousarray(
                outT[i].transpose(1, 0, 2)).reshape(N * UNITS)
    return out
